# revision 56
# baseline (speedup 1.0000x reference)
"""BitLinearPacked distributed Trainium2 kernel (8 NeuronCores).

Problem: out[b, s, o] = sum_i x[b, s, i] * w[o, i]
  with w = unpack_bits(bp) * scale, bits MSB-first, w in {-scale, +scale},
  x: [4, 2048, 4096] f32, bp: [4096*4096/8] int32 (byte values), out f32.

Strategy (token/data parallel — no collectives needed):
  * The 8192 tokens are sharded 8 ways; every core gets the full packed
    weight and computes its tokens' full [1024, 4096] output slab.
  * Mixed-precision contraction: 16 of 32 k-blocks run as fp8-e4m3
    DoubleRow matmuls (2 k-elements per PE cell per cycle; weights
    +-scale are exact in fp8 for scale=1, x is RNE-quantized to e4m3),
    the other 16 in bf16.  KDR=16 is the error-budget cap
    (e4m3 cost ~2.7%*sqrt(KDR/KB) ~ 1.9% < 2e-2).
  * Scheduling (from trace analysis of the previous version):
      - x tiles load as RAW f32 on the sync HWDGE ring (~1.45us/tile
        sustained) instead of casting SWDGE DMAs (~2.2-2.9us/tile) that
        starved the PE during the first bf16 pass and kept the HAM
        clock gate cold for ~15us.
      - bf16 x casts run on DVE, interleaved into chunk-0's unpack
        chain; e4m3 x casts run on ScalarE directly from f32 (single
        rounding), emitted early while ScalarE is otherwise idle.
      - weight unpack (bitwise_and + affine) is split across DVE
        (chunks 0,3,5,7) and GpSimd (chunks 1,2,4,6) so the two chunks
        consumed by each pass unpack in parallel at PE rate; ScalarE
        only does the 16 e4m3 casts + drain copies + DMA posts.
      - ~9 dummy warmup matmuls on a memset tile run before the real
        ones to lift the PE HAM clock gate (4/8 -> 8/8) during the
        input-marshalling window.
      - first bpr slices are 2 k-blocks wide and posted on the idle
        sync ring so the first real matmul issues ~9us in.
  * Host marshalling is pure layout (transpose/reshape/replicate of
    existing values - no arithmetic); packed-weight bytes land so that
    partition p of k-block kb holds byte B[o, kb*16 + p//8], in
    consumption order (bf16 blocks first, then fp8 pairs).
  * chunks 0+1 run as a two-phase superchunk: bf16 partial sums
    for 16 (group, th) banks drain to SBUF while x still streams
    in, then the fp8 phase reuses PSUM and the drain adds the
    partial back.
  * chunks 2..7: one accumulation group (bf16 then DR) per bank,
    two 4-bank passes per chunk so bank sets double-buffer; the final
    chunk runs 2-bank passes to shorten the drain tail.
  * Output is produced transposed ([4096, 1024] per core); the host
    transposes and concatenates the 8 slabs.
"""

from contextlib import ExitStack

import numpy as np

import concourse.bass as bass
import concourse.tile as tile
from concourse import bacc, mybir
from concourse.alu_op_type import AluOpType
from concourse.bass_utils import run_bass_kernel_spmd
from concourse.tile_rust import add_dep_helper

# If a caller forces tracing (BASS_TRACE=1), don't let a missing artifact
# store kill the run — fall back to a local path marker.
import concourse.bass_utils as _bu

_orig_upload = _bu.upload_artifacts


def _safe_upload(tmpdir):
    try:
        return _orig_upload(tmpdir)
    except Exception:
        return f"local:{tmpdir}"


_bu.upload_artifacts = _safe_upload

# ---- problem constants (hardcoded per harness contract) ----
B, S, IF, OF = 4, 2048, 4096, 4096
NCORES = 8
T = B * S // NCORES          # 1024 tokens per core
OC = 512                     # out-feature chunk (weight unpack granularity)
TH = 512                     # token half (matmul rhs width)
KB = IF // 128               # 32 k-blocks
OCN = OF // OC               # 8 chunks
NTH = T // TH                # 2
NOB = OC // 128              # 4

NDR = 8                      # fp8 DoubleRow virtual blocks (256 k each)
KDR = 2 * NDR                # k-blocks handled in fp8
BF_KBS = list(range(KDR, KB))            # bf16 k-blocks (consumed first)
NBFP = len(BF_KBS) // 2                  # fused bf16 k-block pairs
KB_ORDER = BF_KBS + list(range(KDR))     # bpr column order = consumption order

# bpr load piece widths (in k-blocks): chunks 0/1 get small first pieces
# so the first unpack lands fast; later chunks use coarse pieces.
PIECES_FAST = [4, 4, 8, 8, 8]
PIECES_BULK = [8, 8, 8, 8]
HEADW = PIECES_FAST[0]       # replicated head piece width (k-blocks)
NDUMMY = 56                  # HAM warmup matmuls (bridge the ~12us cold
                             # DMA-queue latency with the clock gate warm)


def build_kernel(debug=False):
    nc = bacc.Bacc("TRN2", target_bir_lowering=False, debug=debug)
    dt = mybir.dt

    xt_d = nc.dram_tensor("xt", [IF, T], dt.float32, kind="ExternalInput")
    # bpr: bit-replicated bytes for chunks 0/1 (startup-latency-critical,
    # loaded directly); bprc: compact bytes for chunks 2..7, replicated
    # 16->128 partitions on device by SWDGE broadcast (saves ~10.5MB of
    # HBM reads, which is what paces the x feed in the first ~70us).
    bpr_d = nc.dram_tensor(
        "bpr", [2, 128, KB * OC], dt.int8, kind="ExternalInput"
    )
    bprc_d = nc.dram_tensor(
        "bprc", [OCN, 16, KB * OC], dt.int8, kind="ExternalInput"
    )
    scale_d = nc.dram_tensor("scale", [128], dt.float32, kind="ExternalInput")
    out_d = nc.dram_tensor("out", [OF, T], dt.float32, kind="ExternalOutput")

    with tile.TileContext(nc) as tc, ExitStack() as ctx:
        const_p = ctx.enter_context(tc.tile_pool(name="const", bufs=1))
        xf_p = ctx.enter_context(tc.tile_pool(name="xf", bufs=4))
        xt_p = ctx.enter_context(tc.tile_pool(name="xt", bufs=len(BF_KBS)))
        xq_p = ctx.enter_context(tc.tile_pool(name="xq", bufs=NDR))
        bpr_p = ctx.enter_context(tc.tile_pool(name="bpr", bufs=2))
        t1_p = ctx.enter_context(tc.tile_pool(name="t1", bufs=4))
        wt_p = ctx.enter_context(tc.tile_pool(name="wt", bufs=1))
        part_p = ctx.enter_context(tc.tile_pool(name="part", bufs=16))
        ost_p = ctx.enter_context(tc.tile_pool(name="ost", bufs=4))
        psum_p = ctx.enter_context(
            tc.tile_pool(name="psum", bufs=8, space=bass.MemorySpace.PSUM)
        )

        # ---- HAM warmup: memset a dummy tile on DVE (its first op — no
        # deps), then issue back-to-back dummy matmuls so the PE clock
        # gate lifts to 8/8 while inputs are still marshalling. ----
        dummy = const_p.tile([128, 640], dt.bfloat16)
        nc.vector.memset(dummy[:], 0.03125)
        dummy_ps = psum_p.tile([128, TH], dt.float32, tag="ps", name="dummy_ps")
        for _ in range(NDUMMY):
            nc.tensor.matmul(
                dummy_ps[:], dummy[:, :128], dummy[:, 128:], start=True, stop=True
            )

        # ---- constants, all computed on device ----
        # Bit extraction is a per-partition left shift: partition p shifts
        # its byte by p%8 so bit 7-(p%8) lands at bit 7, then masks
        # 0x80 per byte (int32 lanes; cross-byte spill stays below bit 7).
        # The resulting bytes {0x00, 0x80} read as int8 {0, -128}, so the
        # affine w = t1 * (-s/64) - s gives {-s, +s} with constant scale.
        s_p0 = const_p.tile([1, 1], dt.float32)
        nc.sync.dma_start(
            s_p0[:], scale_d.ap()[0:1].rearrange("(p one) -> p one", one=1)
        )
        iot_t = const_p.tile([128, 1], dt.int32)
        nc.gpsimd.iota(iot_t[:], pattern=[[0, 1]], base=0, channel_multiplier=1)
        qsh_t = const_p.tile([128, 1], dt.int32)
        nc.vector.tensor_scalar(
            qsh_t[:], iot_t[:], 7, None, op0=AluOpType.bitwise_and
        )
        maskc_t = const_p.tile([128, 1], dt.int32)
        nc.vector.memset(maskc_t[:], -2139062144)  # 0x80808080
        # (s-broadcast + scale consts are emitted after the x posts: the
        # scale DMA completes late and must not head-of-line-block the
        # gpsimd post queue)
        s_all = const_p.tile([128, 1], dt.float32)
        negs_t = const_p.tile([128, 1], dt.float32)
        sc_t = const_p.tile([128, 1], dt.float32)

        def emit_scale_consts():
            nc.gpsimd.partition_broadcast(s_all[:], s_p0[:])
            nc.vector.tensor_scalar_mul(negs_t[:], s_all[:], -1.0)
            nc.vector.tensor_scalar_mul(sc_t[:], s_all[:], -1.0 / 64.0)

        # ---- bpr first slices for chunks 0/1 on the idle sync ring ----
        first_piece = {}
        for oc_i in (0, 1):
            piece = bpr_p.tile(
                [128, HEADW * OC], dt.int8, tag=f"bpr{HEADW}", bufs=4,
                name=f"bprf{oc_i}",
            )
            nc.sync.dma_start(piece[:], bpr_d.ap()[oc_i][:, : HEADW * OC])
            first_piece[oc_i] = piece

        # ---- x tiles ----
        # bf16 k-blocks 16..23: raw f32 on the sync HWDGE ring + DVE casts
        # (fast path to feed the first bf16 pass at PE rate); 24..31:
        # SWDGE casting DMAs on the otherwise-idle gpsimd ring (slower
        # per tile but off the critical engines, needed only ~25us in);
        # fp8 k-blocks 0..15: raw f32 on sync + ScalarE e4m3 casts.
        DVE_CAST_KBS = BF_KBS[:11]
        SWDGE_KBS = BF_KBS[11:]
        xf32 = {}
        xt = {}   # bf16 tiles
        ctiles = {}   # compact bpr staging for chunks 2..7

        def post_xf(kb):
            t = xf_p.tile([128, T], dt.float32, tag="xf", name=f"xf{kb}")
            nc.sync.dma_start(t[:], xt_d.ap()[kb * 128 : (kb + 1) * 128, :])
            xf32[kb] = t

        def post_compacts(oc_i):
            tiles = []
            base = 0
            for w in PIECES_BULK:
                ctile = bpr_p.tile(
                    [16, w * OC], dt.int8, tag=f"cbpr{w}", bufs=2, name="cbprc"
                )
                nc.sync.dma_start(
                    ctile[:],
                    bprc_d.ap()[oc_i][:, base * OC : (base + w) * OC],
                )
                tiles.append((ctile, base, w))
                base += w
            ctiles[oc_i] = tiles

        # x f32 posts interleaved with the (tiny) compact bpr loads so
        # chunk 2..7 bytes are staged long before their unpack window
        # without delaying the x tiles they compete with.
        for kb in DVE_CAST_KBS[:2]:
            post_xf(kb)
        for kb in SWDGE_KBS:
            t = xt_p.tile([128, T], dt.bfloat16, tag="xt", name=f"xt{kb}")
            nc.gpsimd.dma_start(
                out=t[:], in_=xt_d.ap()[kb * 128 : (kb + 1) * 128, :]
            )
            xt[kb] = t
        post_compacts(2)
        for kb in DVE_CAST_KBS[2:]:
            post_xf(kb)
        post_compacts(3)
        for kb in range(0, 4):
            post_xf(kb)
        post_compacts(4)
        for kb in range(4, 8):
            post_xf(kb)
        post_compacts(5)
        for kb in range(8, 12):
            post_xf(kb)
        post_compacts(6)
        for kb in range(12, KDR):
            post_xf(kb)
        post_compacts(7)
        emit_scale_consts()

        # broadcast posts (gpsimd SWDGE): replicate compact rows
        # 16 -> 128 partitions; paced by pool-slot WARs automatically.
        pre_pieces = {}
        for oc_i in range(2, OCN):
            plist = []
            for (ctile, base, w) in ctiles[oc_i]:
                piece = bpr_p.tile(
                    [128, w * OC], dt.int8, tag=f"bbpr{w}", bufs=3,
                    name="bbprc",
                )
                nc.gpsimd.dma_start(
                    piece[:],
                    ctile[:].unsqueeze(1).broadcast_to((16, 8, w * OC)),
                )
                plist.append((piece, base, w))
            pre_pieces[oc_i] = plist

        xq = []   # e4m3 DoubleRow pair tiles, filled by ScalarE casts
        for v in range(NDR):
            xq.append(xq_p.tile([128, 2, T], dt.float8e4, tag="xq", name=f"xq{v}"))

        # ---- per out-feature chunk: unpack weights (2 k-blocks per op) ----
        # Shift-unpacks always run on DVE; the affine runs on aff_eng.
        # Pieces load compact [16, w*OC] on the scalar HWDGE ring, then a
        # gpsimd SWDGE SBUF->SBUF DMA with a 0-stride broadcast AP
        # replicates row q to partitions 8q..8q+7.
        def post_piece(oc_i, base, w, first=None):
            if first is not None:
                return (first, base, w)
            piece = bpr_p.tile(
                [128, w * OC], dt.int8, tag=f"bpr{w}",
                bufs=(4 if w == 8 else 4), name="bprpc"
            )
            nc.scalar.dma_start(
                piece[:], bpr_d.ap()[oc_i][:, base * OC : (base + w) * OC]
            )
            return (piece, base, w)

        def pair_src(pieces, f):
            j = 2 * f
            for piece, pbase, pw in pieces:
                if pbase <= j and j + 2 <= pbase + pw:
                    return piece[:, (j - pbase) * OC : (j - pbase + 2) * OC]
            raise AssertionError("piece widths must cover even pairs")

        def emit_pair(oc_i, f, pieces, aff_eng, wt2, wq):
            t1 = t1_p.tile([128, 2 * OC], dt.int8, tag="t1", name="t1c")
            # per-partition shift + 0x80-mask in int32 lanes (DVE bitwise
            # fast path): bytes become {0x00, 0x80} = int8 {0, -128}
            shift_inst = nc.vector.tensor_scalar(
                t1[:].bitcast(dt.int32),
                pair_src(pieces, f).bitcast(dt.int32),
                qsh_t[:], maskc_t[:],
                op0=AluOpType.logical_shift_left, op1=AluOpType.bitwise_and,
            )
            # w = t1 * (-s/64) - s  ->  {-s, +s}
            if f < NBFP:
                wt = wt_p.tile(
                    [128, 2 * OC], dt.bfloat16, tag="wtb", bufs=2 * NBFP,
                    name="wtc",
                )
                wt2[f] = wt
                dst = wt[:]
            else:
                v = f - NBFP
                wq[v] = wt_p.tile(
                    [128, 2, OC], dt.float8e4, tag="wtq", bufs=2 * NDR,
                    name="wqc",
                )
                dst = wq[v][:, :, :]
            if aff_eng is nc.scalar:
                nc.scalar.activation(
                    dst, t1[:],
                    mybir.ActivationFunctionType.Identity,
                    bias=negs_t[:], scale=sc_t[:],
                )
            else:
                aff_eng.tensor_scalar(
                    dst, t1[:], sc_t[:], negs_t[:],
                    op0=AluOpType.mult, op1=AluOpType.add,
                )
            return shift_inst

        def emit_unpack(oc_i, aff_eng):
            pieces = pre_pieces[oc_i]
            wt2, wq = {}, {}
            for f in range(KB // 2):
                emit_pair(oc_i, f, pieces, aff_eng, wt2, wq)
            return wt2, wq

        def emit_unpack_superchunk():
            """Chunks 0+1 fused: per pair f, both shift-unpacks on DVE,
            chunk0's affine on ScalarE, chunk1's on DVE, and the first 8
            bf16 x casts on DVE — interleaved so weight and x delivery
            track PE consumption from the first matmul on.  Later bpr
            pieces are posted from inside the loop, and the x flood is
            released only after the first unpack so the startup-critical
            transfers are not starved by queue round-robin."""
            p0 = [post_piece(0, 0, 4, first=first_piece[0])]
            p1 = [post_piece(1, 0, 4, first=first_piece[1])]
            p0.append(post_piece(0, 4, 4))
            p1.append(post_piece(1, 4, 4))
            w0, q0 = {}, {}
            w1, q1 = {}, {}
            for f in range(KB // 2):
                inst0 = emit_pair(0, f, p0, nc.scalar, w0, q0)
                emit_pair(1, f, p1, nc.vector, w1, q1)
                if f < NBFP:
                    for kb in (BF_KBS[2 * f], BF_KBS[2 * f + 1]):
                        if kb not in DVE_CAST_KBS:
                            continue
                        xtile = xt_p.tile(
                            [128, T], dt.bfloat16, tag="xt", name=f"xt{kb}"
                        )
                        nc.vector.tensor_copy(xtile[:], xf32[kb][:])
                        xt[kb] = xtile
                if f == 1:
                    p0.append(post_piece(0, 8, 8))
                    p1.append(post_piece(1, 8, 8))
                elif f == 5:
                    p0.append(post_piece(0, 16, 8))
                    p1.append(post_piece(1, 16, 8))
                elif f == 9:
                    p0.append(post_piece(0, 24, 8))
                    p1.append(post_piece(1, 24, 8))
            return (w0, q0), (w1, q1)

        def emit_xq_casts():
            # RNE casts f32 -> e4m3 on ScalarE (single rounding).
            for v in range(NDR):
                for i in range(2):
                    nc.scalar.activation(
                        xq[v][:, i : i + 1, :],
                        xf32[2 * v + i][:],
                        mybir.ActivationFunctionType.Identity,
                    )

        def bf_lhsT(weights, oc_i, kb, ob):
            f, half = (kb - KDR) // 2, (kb - KDR) % 2
            wt = weights[oc_i][0][f]
            c0 = half * OC + ob * 128
            return wt[:, c0 : c0 + 128]

        # ---- matmul passes ----
        # groups: list of (oc_i, ob); one PSUM bank per (group, th).
        # phase: "full" = bf16+DR one group, drain to out;
        #        "bf"   = bf16 only, drain to partial tiles (returned);
        #        "dr"   = DR only from fresh PSUM, drain adds partial.
        def emit_pass(groups, weights, phase="full", partials=None):
            pss = {}
            for g in groups:
                for th in range(NTH):
                    ps = psum_p.tile([128, TH], dt.float32, tag="ps", name="ps")
                    pss[(g, th)] = ps
            if phase in ("full", "bf"):
                for idx, kb in enumerate(BF_KBS):
                    for (oc_i, ob) in groups:
                        lhsT = bf_lhsT(weights, oc_i, kb, ob)
                        for th in range(NTH):
                            nc.tensor.matmul(
                                pss[((oc_i, ob), th)][:],
                                lhsT,
                                xt[kb][:, th * TH : (th + 1) * TH],
                                start=(idx == 0),
                                stop=(phase == "bf" and idx == len(BF_KBS) - 1),
                            )
            if phase in ("full", "dr"):
                for v in range(NDR):
                    for (oc_i, ob) in groups:
                        lhsT = weights[oc_i][1][v][:, :, ob * 128 : (ob + 1) * 128]
                        for th in range(NTH):
                            nc.tensor.matmul(
                                pss[((oc_i, ob), th)][:],
                                lhsT,
                                xq[v][:, :, th * TH : (th + 1) * TH],
                                start=(phase == "dr" and v == 0),
                                stop=(v == NDR - 1),
                                perf_mode=mybir.MatmulPerfMode.DoubleRow,
                            )
            out_parts = {}
            for gi, g in enumerate(groups):
                oc_i, ob = g
                o0 = oc_i * OC + ob * 128
                for th in range(NTH):
                    if phase == "bf":
                        pt = part_p.tile(
                            [128, TH], dt.float32, tag="part", name="partc"
                        )
                        nc.vector.tensor_copy(pt[:], pss[(g, th)][:])
                        out_parts[(g, th)] = pt
                        continue
                    st = ost_p.tile([128, TH], dt.float32, tag="ost", name="st")
                    if phase == "dr":
                        nc.vector.tensor_tensor(
                            st[:], pss[(g, th)][:], partials[(g, th)][:],
                            op=AluOpType.add,
                        )
                    else:
                        nc.scalar.activation(
                            st[:], pss[(g, th)][:],
                            mybir.ActivationFunctionType.Identity,
                        )
                    eng = nc.scalar if (gi + th) % 2 == 0 else nc.sync
                    eng.dma_start(
                        out_d.ap()[o0 : o0 + 128, th * TH : (th + 1) * TH],
                        st[:],
                    )
            return out_parts

        UNPACK_ENG = {
            2: nc.scalar, 3: nc.scalar, 4: nc.scalar, 5: nc.vector,
            6: nc.scalar, 7: nc.vector,
        }
        weights = {}
        weights[0], weights[1] = emit_unpack_superchunk()
        emit_xq_casts()
        # superchunk: chunks 0+1, two-phase (bf16 partials, then DR + add)
        gA = [(0, 0), (0, 1), (1, 0), (1, 1)]
        gB = [(0, 2), (0, 3), (1, 2), (1, 3)]
        pA = emit_pass(gA, weights, phase="bf")
        pB = emit_pass(gB, weights, phase="bf")
        # chunk-2 unpack on GpSimd (free after chunk 1) before the DR
        # phases so its chain runs during them.
        weights[2] = emit_unpack(2, UNPACK_ENG[2])
        emit_pass(gA, weights, phase="dr", partials=pA)
        emit_pass(gB, weights, phase="dr", partials=pB)
        emit_pass([(2, 0), (2, 1)], weights, phase="full")
        weights[3] = emit_unpack(3, UNPACK_ENG[3])
        emit_pass([(2, 2), (2, 3)], weights, phase="full")
        for oc_i in range(3, OCN):
            if oc_i + 1 < OCN:
                weights[oc_i + 1] = emit_unpack(
                    oc_i + 1, UNPACK_ENG[oc_i + 1]
                )
            if oc_i == OCN - 1:
                for ob in range(NOB):
                    emit_pass([(oc_i, ob)], weights, phase="full")
            else:
                for ob0 in range(0, NOB, 2):
                    emit_pass(
                        [(oc_i, ob0), (oc_i, ob0 + 1)], weights, phase="full"
                    )
            del weights[oc_i]

    nc.compile()
    return nc


def marshal_bpr(bp_u8_mat):
    """bp_u8_mat: [O, I//8] u8. Returns (bprh, bprc):
    bprc[oc, q, j*OC + o] = B[oc*OC + o, KB_ORDER[j]*16 + q] (compact);
    bprh = bprc[:2] bit-replicated to 128 partitions (p -> row p//8)."""
    O, JJ = bp_u8_mat.shape
    KB_ = JJ // 16
    OCN_ = O // OC
    Bt = np.ascontiguousarray(bp_u8_mat.T).reshape(KB_, 16, O)
    Bt = Bt[np.array(KB_ORDER)]     # consumption order
    bprc = (
        Bt.reshape(KB_, 16, OCN_, OC)
        .transpose(2, 1, 0, 3)
        .reshape(OCN_, 16, KB_ * OC)
    )
    bprc = np.ascontiguousarray(bprc).view(np.int8)
    bprh = np.ascontiguousarray(np.repeat(bprc[:2], 8, axis=1))
    return bprh, bprc


def make_in_maps(x, bp, scale):
    """Host-side marshalling (layout only): token-shard + transpose x,
    byte-shuffle bp, replicate scale."""
    x = np.asarray(x, dtype=np.float32).reshape(B * S, IF)
    sval = np.float32(np.asarray(scale, dtype=np.float32).reshape(-1)[0])
    bprh, bprc = marshal_bpr(
        np.asarray(bp).astype(np.uint8).reshape(OF, IF // 8)
    )
    scale_rep = np.full((128,), sval, dtype=np.float32)
    return [
        {
            "xt": np.ascontiguousarray(x[c * T : (c + 1) * T].T),
            "bpr": bprh,
            "bprc": bprc,
            "scale": scale_rep,
        }
        for c in range(NCORES)
    ]


_NC_CACHE = None


def _get_nc():
    global _NC_CACHE
    if _NC_CACHE is None:
        _NC_CACHE = build_kernel()
    return _NC_CACHE


def gather(results):
    out = np.concatenate([results[c]["out"].T for c in range(NCORES)], axis=0)
    return np.ascontiguousarray(out.reshape(B, S, OF).astype(np.float32))


def kernel(x, bp, scale):
    in_maps = make_in_maps(x, bp, scale)
    nc = _get_nc()
    res = run_bass_kernel_spmd(nc, in_maps, core_ids=list(range(NCORES)))
    return gather(res.results)


if __name__ == "__main__":
    rng = np.random.default_rng(0)
    x = rng.standard_normal((B, S, IF), dtype=np.float32)
    bp = rng.integers(0, 256, size=(OF * IF // 8,), dtype=np.int32)
    scale = np.ones((1,), dtype=np.float32)
    out = kernel(x=x, bp=bp, scale=scale)
    print(out.shape, out.dtype)


# revision 57
# speedup vs baseline: 1.0149x; 1.0149x over previous
"""BitLinearPacked distributed Trainium2 kernel (8 NeuronCores).

Problem: out[b, s, o] = sum_i x[b, s, i] * w[o, i]
  with w = unpack_bits(bp) * scale, bits MSB-first, w in {-scale, +scale},
  x: [4, 2048, 4096] f32, bp: [4096*4096/8] int32 (byte values), out f32.

Strategy (token/data parallel — no collectives needed):
  * The 8192 tokens are sharded 8 ways; every core gets the full packed
    weight and computes its tokens' full [1024, 4096] output slab.
  * Mixed-precision contraction: 16 of 32 k-blocks run as fp8-e4m3
    DoubleRow matmuls (2 k-elements per PE cell per cycle; weights
    +-scale are exact in fp8 for scale=1, x is RNE-quantized to e4m3),
    the other 16 in bf16.  KDR=16 is the error-budget cap
    (e4m3 cost ~2.7%*sqrt(KDR/KB) ~ 1.9% < 2e-2).
  * Scheduling (from trace analysis of the previous version):
      - x tiles load as RAW f32 on the sync HWDGE ring (~1.45us/tile
        sustained) instead of casting SWDGE DMAs (~2.2-2.9us/tile) that
        starved the PE during the first bf16 pass and kept the HAM
        clock gate cold for ~15us.
      - bf16 x casts run on DVE, interleaved into chunk-0's unpack
        chain; e4m3 x casts run on ScalarE directly from f32 (single
        rounding), emitted early while ScalarE is otherwise idle.
      - weight unpack (bitwise_and + affine) is split across DVE
        (chunks 0,3,5,7) and GpSimd (chunks 1,2,4,6) so the two chunks
        consumed by each pass unpack in parallel at PE rate; ScalarE
        only does the 16 e4m3 casts + drain copies + DMA posts.
      - ~9 dummy warmup matmuls on a memset tile run before the real
        ones to lift the PE HAM clock gate (4/8 -> 8/8) during the
        input-marshalling window.
      - first bpr slices are 2 k-blocks wide and posted on the idle
        sync ring so the first real matmul issues ~9us in.
  * Host marshalling is pure layout (transpose/reshape/replicate of
    existing values - no arithmetic); packed-weight bytes land so that
    partition p of k-block kb holds byte B[o, kb*16 + p//8], in
    consumption order (bf16 blocks first, then fp8 pairs).
  * chunks 0+1 run as a two-phase superchunk: bf16 partial sums
    for 16 (group, th) banks drain to SBUF while x still streams
    in, then the fp8 phase reuses PSUM and the drain adds the
    partial back.
  * chunks 2..7: one accumulation group (bf16 then DR) per bank,
    two 4-bank passes per chunk so bank sets double-buffer; the final
    chunk runs 2-bank passes to shorten the drain tail.
  * Output is produced transposed ([4096, 1024] per core); the host
    transposes and concatenates the 8 slabs.
"""

from contextlib import ExitStack

import numpy as np

import concourse.bass as bass
import concourse.tile as tile
from concourse import bacc, mybir
from concourse.alu_op_type import AluOpType
from concourse.bass_utils import run_bass_kernel_spmd
from concourse.tile_rust import add_dep_helper

# If a caller forces tracing (BASS_TRACE=1), don't let a missing artifact
# store kill the run — fall back to a local path marker.
import concourse.bass_utils as _bu

_orig_upload = _bu.upload_artifacts


def _safe_upload(tmpdir):
    try:
        return _orig_upload(tmpdir)
    except Exception:
        return f"local:{tmpdir}"


_bu.upload_artifacts = _safe_upload

# ---- problem constants (hardcoded per harness contract) ----
B, S, IF, OF = 4, 2048, 4096, 4096
NCORES = 8
T = B * S // NCORES          # 1024 tokens per core
OC = 512                     # out-feature chunk (weight unpack granularity)
TH = 512                     # token half (matmul rhs width)
KB = IF // 128               # 32 k-blocks
OCN = OF // OC               # 8 chunks
NTH = T // TH                # 2
NOB = OC // 128              # 4

NDR = 8                      # fp8 DoubleRow virtual blocks (256 k each)
KDR = 2 * NDR                # k-blocks handled in fp8
BF_KBS = list(range(KDR, KB))            # bf16 k-blocks (consumed first)
NBFP = len(BF_KBS) // 2                  # fused bf16 k-block pairs
KB_ORDER = BF_KBS + list(range(KDR))     # bpr column order = consumption order

# bpr load piece widths (in k-blocks): chunks 0/1 get small first pieces
# so the first unpack lands fast; later chunks use coarse pieces.
PIECES_FAST = [4, 4, 8, 8, 8]
PIECES_BULK = [8, 8, 8, 8]
HEADW = PIECES_FAST[0]       # replicated head piece width (k-blocks)
NDUMMY = 56                  # HAM warmup matmuls (bridge the ~12us cold
                             # DMA-queue latency with the clock gate warm)


def build_kernel(debug=False):
    nc = bacc.Bacc("TRN2", target_bir_lowering=False, debug=debug)
    dt = mybir.dt

    xt_d = nc.dram_tensor("xt", [IF, T], dt.float32, kind="ExternalInput")
    # bpr: bit-replicated bytes for chunks 0/1 (startup-latency-critical,
    # loaded directly); bprc: compact bytes for chunks 2..7, replicated
    # 16->128 partitions on device by SWDGE broadcast (saves ~10.5MB of
    # HBM reads, which is what paces the x feed in the first ~70us).
    bpr_d = nc.dram_tensor(
        "bpr", [2, 128, KB * OC], dt.int8, kind="ExternalInput"
    )
    bprc_d = nc.dram_tensor(
        "bprc", [OCN, 16, KB * OC], dt.int8, kind="ExternalInput"
    )
    scale_d = nc.dram_tensor("scale", [128], dt.float32, kind="ExternalInput")
    out_d = nc.dram_tensor("out", [OF, T], dt.float32, kind="ExternalOutput")

    with tile.TileContext(nc) as tc, ExitStack() as ctx:
        const_p = ctx.enter_context(tc.tile_pool(name="const", bufs=1))
        xf_p = ctx.enter_context(tc.tile_pool(name="xf", bufs=4))
        xt_p = ctx.enter_context(tc.tile_pool(name="xt", bufs=len(BF_KBS)))
        xq_p = ctx.enter_context(tc.tile_pool(name="xq", bufs=NDR))
        bpr_p = ctx.enter_context(tc.tile_pool(name="bpr", bufs=2))
        t1_p = ctx.enter_context(tc.tile_pool(name="t1", bufs=4))
        wt_p = ctx.enter_context(tc.tile_pool(name="wt", bufs=1))
        part_p = ctx.enter_context(tc.tile_pool(name="part", bufs=16))
        ost_p = ctx.enter_context(tc.tile_pool(name="ost", bufs=4))
        psum_p = ctx.enter_context(
            tc.tile_pool(name="psum", bufs=8, space=bass.MemorySpace.PSUM)
        )

        # ---- HAM warmup: memset a dummy tile on DVE (its first op — no
        # deps), then issue back-to-back dummy matmuls so the PE clock
        # gate lifts to 8/8 while inputs are still marshalling. ----
        dummy = const_p.tile([128, 640], dt.bfloat16)
        nc.vector.memset(dummy[:], 0.03125)
        dummy_ps = psum_p.tile([128, TH], dt.float32, tag="ps", name="dummy_ps")
        for _ in range(NDUMMY):
            nc.tensor.matmul(
                dummy_ps[:], dummy[:, :128], dummy[:, 128:], start=True, stop=True
            )

        # ---- constants, all computed on device ----
        # Bit extraction is a per-partition left shift: partition p shifts
        # its byte by p%8 so bit 7-(p%8) lands at bit 7, then masks
        # 0x80 per byte (int32 lanes; cross-byte spill stays below bit 7).
        # The resulting bytes {0x00, 0x80} read as int8 {0, -128}, so the
        # affine w = t1 * (-s/64) - s gives {-s, +s} with constant scale.
        s_p0 = const_p.tile([1, 1], dt.float32)
        nc.sync.dma_start(
            s_p0[:], scale_d.ap()[0:1].rearrange("(p one) -> p one", one=1)
        )
        iot_t = const_p.tile([128, 1], dt.int32)
        nc.gpsimd.iota(iot_t[:], pattern=[[0, 1]], base=0, channel_multiplier=1)
        qsh_t = const_p.tile([128, 1], dt.int32)
        nc.vector.tensor_scalar(
            qsh_t[:], iot_t[:], 7, None, op0=AluOpType.bitwise_and
        )
        maskc_t = const_p.tile([128, 1], dt.int32)
        nc.vector.memset(maskc_t[:], -2139062144)  # 0x80808080
        # (s-broadcast + scale consts are emitted after the x posts: the
        # scale DMA completes late and must not head-of-line-block the
        # gpsimd post queue)
        s_all = const_p.tile([128, 1], dt.float32)
        negs_t = const_p.tile([128, 1], dt.float32)
        sc_t = const_p.tile([128, 1], dt.float32)

        def emit_scale_consts():
            nc.gpsimd.partition_broadcast(s_all[:], s_p0[:])
            nc.vector.tensor_scalar_mul(negs_t[:], s_all[:], -1.0)
            nc.vector.tensor_scalar_mul(sc_t[:], s_all[:], -1.0 / 64.0)

        # ---- bpr first slices for chunks 0/1 on the idle sync ring ----
        first_piece = {}
        for oc_i in (0, 1):
            piece = bpr_p.tile(
                [128, HEADW * OC], dt.int8, tag=f"bpr{HEADW}", bufs=4,
                name=f"bprf{oc_i}",
            )
            nc.sync.dma_start(piece[:], bpr_d.ap()[oc_i][:, : HEADW * OC])
            first_piece[oc_i] = piece

        # ---- x tiles ----
        # bf16 k-blocks 16..23: raw f32 on the sync HWDGE ring + DVE casts
        # (fast path to feed the first bf16 pass at PE rate); 24..31:
        # SWDGE casting DMAs on the otherwise-idle gpsimd ring (slower
        # per tile but off the critical engines, needed only ~25us in);
        # fp8 k-blocks 0..15: raw f32 on sync + ScalarE e4m3 casts.
        DVE_CAST_KBS = BF_KBS
        SWDGE_KBS = []
        xf32 = {}
        xt = {}   # bf16 tiles
        ctiles = {}   # compact bpr staging for chunks 2..7

        def post_xf(kb):
            t = xf_p.tile([128, T], dt.float32, tag="xf", name=f"xf{kb}")
            nc.sync.dma_start(t[:], xt_d.ap()[kb * 128 : (kb + 1) * 128, :])
            xf32[kb] = t

        def post_compacts(oc_i):
            tiles = []
            base = 0
            for w in PIECES_BULK:
                ctile = bpr_p.tile(
                    [16, w * OC], dt.int8, tag=f"cbpr{w}", bufs=2, name="cbprc"
                )
                nc.sync.dma_start(
                    ctile[:],
                    bprc_d.ap()[oc_i][:, base * OC : (base + w) * OC],
                )
                tiles.append((ctile, base, w))
                base += w
            ctiles[oc_i] = tiles

        # x f32 posts interleaved with the (tiny) compact bpr loads so
        # chunk 2..7 bytes are staged long before their unpack window
        # without delaying the x tiles they compete with.
        for kb in DVE_CAST_KBS[:8]:
            post_xf(kb)
        post_compacts(2)
        for kb in DVE_CAST_KBS[8:]:
            post_xf(kb)
        post_compacts(3)
        for kb in range(0, 4):
            post_xf(kb)
        post_compacts(4)
        for kb in range(4, 8):
            post_xf(kb)
        post_compacts(5)
        for kb in range(8, 12):
            post_xf(kb)
        post_compacts(6)
        for kb in range(12, KDR):
            post_xf(kb)
        post_compacts(7)
        emit_scale_consts()

        # broadcast posts (gpsimd SWDGE): replicate compact rows
        # 16 -> 128 partitions; paced by pool-slot WARs automatically.
        pre_pieces = {}
        for oc_i in range(2, OCN):
            plist = []
            for (ctile, base, w) in ctiles[oc_i]:
                piece = bpr_p.tile(
                    [128, w * OC], dt.int8, tag=f"bbpr{w}", bufs=3,
                    name="bbprc",
                )
                nc.gpsimd.dma_start(
                    piece[:],
                    ctile[:].unsqueeze(1).broadcast_to((16, 8, w * OC)),
                )
                plist.append((piece, base, w))
            pre_pieces[oc_i] = plist

        xq = []   # e4m3 DoubleRow pair tiles, filled by ScalarE casts
        for v in range(NDR):
            xq.append(xq_p.tile([128, 2, T], dt.float8e4, tag="xq", name=f"xq{v}"))

        # ---- per out-feature chunk: unpack weights (2 k-blocks per op) ----
        # Shift-unpacks always run on DVE; the affine runs on aff_eng.
        # Pieces load compact [16, w*OC] on the scalar HWDGE ring, then a
        # gpsimd SWDGE SBUF->SBUF DMA with a 0-stride broadcast AP
        # replicates row q to partitions 8q..8q+7.
        def post_piece(oc_i, base, w, first=None):
            if first is not None:
                return (first, base, w)
            piece = bpr_p.tile(
                [128, w * OC], dt.int8, tag=f"bpr{w}",
                bufs=(4 if w == 8 else 4), name="bprpc"
            )
            nc.scalar.dma_start(
                piece[:], bpr_d.ap()[oc_i][:, base * OC : (base + w) * OC]
            )
            return (piece, base, w)

        def pair_src(pieces, f):
            j = 2 * f
            for piece, pbase, pw in pieces:
                if pbase <= j and j + 2 <= pbase + pw:
                    return piece[:, (j - pbase) * OC : (j - pbase + 2) * OC]
            raise AssertionError("piece widths must cover even pairs")

        def emit_pair(oc_i, f, pieces, aff_eng, wt2, wq):
            t1 = t1_p.tile([128, 2 * OC], dt.int8, tag="t1", name="t1c")
            # per-partition shift + 0x80-mask in int32 lanes (DVE bitwise
            # fast path): bytes become {0x00, 0x80} = int8 {0, -128}
            shift_inst = nc.vector.tensor_scalar(
                t1[:].bitcast(dt.int32),
                pair_src(pieces, f).bitcast(dt.int32),
                qsh_t[:], maskc_t[:],
                op0=AluOpType.logical_shift_left, op1=AluOpType.bitwise_and,
            )
            # w = t1 * (-s/64) - s  ->  {-s, +s}
            if f < NBFP:
                wt = wt_p.tile(
                    [128, 2 * OC], dt.bfloat16, tag="wtb", bufs=2 * NBFP,
                    name="wtc",
                )
                wt2[f] = wt
                dst = wt[:]
            else:
                v = f - NBFP
                wq[v] = wt_p.tile(
                    [128, 2, OC], dt.float8e4, tag="wtq", bufs=2 * NDR,
                    name="wqc",
                )
                dst = wq[v][:, :, :]
            if aff_eng is nc.scalar:
                nc.scalar.activation(
                    dst, t1[:],
                    mybir.ActivationFunctionType.Identity,
                    bias=negs_t[:], scale=sc_t[:],
                )
            else:
                aff_eng.tensor_scalar(
                    dst, t1[:], sc_t[:], negs_t[:],
                    op0=AluOpType.mult, op1=AluOpType.add,
                )
            return shift_inst

        def emit_unpack(oc_i, aff_eng):
            pieces = pre_pieces[oc_i]
            wt2, wq = {}, {}
            for f in range(KB // 2):
                emit_pair(oc_i, f, pieces, aff_eng, wt2, wq)
            return wt2, wq

        def emit_unpack_superchunk():
            """Chunks 0+1 fused: per pair f, both shift-unpacks on DVE,
            chunk0's affine on ScalarE, chunk1's on DVE, and the first 8
            bf16 x casts on DVE — interleaved so weight and x delivery
            track PE consumption from the first matmul on.  Later bpr
            pieces are posted from inside the loop, and the x flood is
            released only after the first unpack so the startup-critical
            transfers are not starved by queue round-robin."""
            p0 = [post_piece(0, 0, 4, first=first_piece[0])]
            p1 = [post_piece(1, 0, 4, first=first_piece[1])]
            p0.append(post_piece(0, 4, 4))
            p1.append(post_piece(1, 4, 4))
            w0, q0 = {}, {}
            w1, q1 = {}, {}
            for f in range(KB // 2):
                emit_pair(0, f, p0, nc.scalar, w0, q0)
                emit_pair(1, f, p1, nc.scalar, w1, q1)
                if f < NBFP:
                    for kb in (BF_KBS[2 * f], BF_KBS[2 * f + 1]):
                        if kb not in DVE_CAST_KBS:
                            continue
                        xtile = xt_p.tile(
                            [128, T], dt.bfloat16, tag="xt", name=f"xt{kb}"
                        )
                        nc.vector.tensor_copy(xtile[:], xf32[kb][:])
                        xt[kb] = xtile
                if f == 1:
                    p0.append(post_piece(0, 8, 8))
                    p1.append(post_piece(1, 8, 8))
                elif f == 5:
                    p0.append(post_piece(0, 16, 8))
                    p1.append(post_piece(1, 16, 8))
                elif f == 9:
                    p0.append(post_piece(0, 24, 8))
                    p1.append(post_piece(1, 24, 8))
            return (w0, q0), (w1, q1)

        def emit_xq_casts():
            # RNE casts f32 -> e4m3 on ScalarE (single rounding).
            for v in range(NDR):
                for i in range(2):
                    nc.scalar.activation(
                        xq[v][:, i : i + 1, :],
                        xf32[2 * v + i][:],
                        mybir.ActivationFunctionType.Identity,
                    )

        def bf_lhsT(weights, oc_i, kb, ob):
            f, half = (kb - KDR) // 2, (kb - KDR) % 2
            wt = weights[oc_i][0][f]
            c0 = half * OC + ob * 128
            return wt[:, c0 : c0 + 128]

        # ---- matmul passes ----
        # groups: list of (oc_i, ob); one PSUM bank per (group, th).
        # phase: "full" = bf16+DR one group, drain to out;
        #        "bf"   = bf16 only, drain to partial tiles (returned);
        #        "dr"   = DR only from fresh PSUM, drain adds partial.
        def emit_pass(groups, weights, phase="full", partials=None):
            pss = {}
            for g in groups:
                for th in range(NTH):
                    ps = psum_p.tile([128, TH], dt.float32, tag="ps", name="ps")
                    pss[(g, th)] = ps
            if phase in ("full", "bf"):
                for idx, kb in enumerate(BF_KBS):
                    for (oc_i, ob) in groups:
                        lhsT = bf_lhsT(weights, oc_i, kb, ob)
                        for th in range(NTH):
                            nc.tensor.matmul(
                                pss[((oc_i, ob), th)][:],
                                lhsT,
                                xt[kb][:, th * TH : (th + 1) * TH],
                                start=(idx == 0),
                                stop=(phase == "bf" and idx == len(BF_KBS) - 1),
                            )
            if phase in ("full", "dr"):
                for v in range(NDR):
                    for (oc_i, ob) in groups:
                        lhsT = weights[oc_i][1][v][:, :, ob * 128 : (ob + 1) * 128]
                        for th in range(NTH):
                            nc.tensor.matmul(
                                pss[((oc_i, ob), th)][:],
                                lhsT,
                                xq[v][:, :, th * TH : (th + 1) * TH],
                                start=(phase == "dr" and v == 0),
                                stop=(v == NDR - 1),
                                perf_mode=mybir.MatmulPerfMode.DoubleRow,
                            )
            out_parts = {}
            for gi, g in enumerate(groups):
                oc_i, ob = g
                o0 = oc_i * OC + ob * 128
                for th in range(NTH):
                    if phase == "bf":
                        pt = part_p.tile(
                            [128, TH], dt.float32, tag="part", name="partc"
                        )
                        nc.vector.tensor_copy(pt[:], pss[(g, th)][:])
                        out_parts[(g, th)] = pt
                        continue
                    st = ost_p.tile([128, TH], dt.float32, tag="ost", name="st")
                    if phase == "dr":
                        nc.vector.tensor_tensor(
                            st[:], pss[(g, th)][:], partials[(g, th)][:],
                            op=AluOpType.add,
                        )
                    else:
                        nc.scalar.activation(
                            st[:], pss[(g, th)][:],
                            mybir.ActivationFunctionType.Identity,
                        )
                    eng = nc.scalar if (gi + th) % 2 == 0 else nc.sync
                    eng.dma_start(
                        out_d.ap()[o0 : o0 + 128, th * TH : (th + 1) * TH],
                        st[:],
                    )
            return out_parts

        UNPACK_ENG = {
            2: nc.scalar, 3: nc.scalar, 4: nc.scalar, 5: nc.vector,
            6: nc.scalar, 7: nc.vector,
        }
        weights = {}
        weights[0], weights[1] = emit_unpack_superchunk()
        emit_xq_casts()
        # superchunk: chunks 0+1, two-phase (bf16 partials, then DR + add)
        gA = [(0, 0), (0, 1), (1, 0), (1, 1)]
        gB = [(0, 2), (0, 3), (1, 2), (1, 3)]
        pA = emit_pass(gA, weights, phase="bf")
        pB = emit_pass(gB, weights, phase="bf")
        # chunk-2 unpack on GpSimd (free after chunk 1) before the DR
        # phases so its chain runs during them.
        weights[2] = emit_unpack(2, UNPACK_ENG[2])
        emit_pass(gA, weights, phase="dr", partials=pA)
        emit_pass(gB, weights, phase="dr", partials=pB)
        emit_pass([(2, 0), (2, 1)], weights, phase="full")
        weights[3] = emit_unpack(3, UNPACK_ENG[3])
        emit_pass([(2, 2), (2, 3)], weights, phase="full")
        for oc_i in range(3, OCN):
            if oc_i + 1 < OCN:
                weights[oc_i + 1] = emit_unpack(
                    oc_i + 1, UNPACK_ENG[oc_i + 1]
                )
            if oc_i == OCN - 1:
                for ob in range(NOB):
                    emit_pass([(oc_i, ob)], weights, phase="full")
            else:
                for ob0 in range(0, NOB, 2):
                    emit_pass(
                        [(oc_i, ob0), (oc_i, ob0 + 1)], weights, phase="full"
                    )
            del weights[oc_i]

    nc.compile()
    return nc


def marshal_bpr(bp_u8_mat):
    """bp_u8_mat: [O, I//8] u8. Returns (bprh, bprc):
    bprc[oc, q, j*OC + o] = B[oc*OC + o, KB_ORDER[j]*16 + q] (compact);
    bprh = bprc[:2] bit-replicated to 128 partitions (p -> row p//8)."""
    O, JJ = bp_u8_mat.shape
    KB_ = JJ // 16
    OCN_ = O // OC
    Bt = np.ascontiguousarray(bp_u8_mat.T).reshape(KB_, 16, O)
    Bt = Bt[np.array(KB_ORDER)]     # consumption order
    bprc = (
        Bt.reshape(KB_, 16, OCN_, OC)
        .transpose(2, 1, 0, 3)
        .reshape(OCN_, 16, KB_ * OC)
    )
    bprc = np.ascontiguousarray(bprc).view(np.int8)
    bprh = np.ascontiguousarray(np.repeat(bprc[:2], 8, axis=1))
    return bprh, bprc


def make_in_maps(x, bp, scale):
    """Host-side marshalling (layout only): token-shard + transpose x,
    byte-shuffle bp, replicate scale."""
    x = np.asarray(x, dtype=np.float32).reshape(B * S, IF)
    sval = np.float32(np.asarray(scale, dtype=np.float32).reshape(-1)[0])
    bprh, bprc = marshal_bpr(
        np.asarray(bp).astype(np.uint8).reshape(OF, IF // 8)
    )
    scale_rep = np.full((128,), sval, dtype=np.float32)
    return [
        {
            "xt": np.ascontiguousarray(x[c * T : (c + 1) * T].T),
            "bpr": bprh,
            "bprc": bprc,
            "scale": scale_rep,
        }
        for c in range(NCORES)
    ]


_NC_CACHE = None


def _get_nc():
    global _NC_CACHE
    if _NC_CACHE is None:
        _NC_CACHE = build_kernel()
    return _NC_CACHE


def gather(results):
    out = np.concatenate([results[c]["out"].T for c in range(NCORES)], axis=0)
    return np.ascontiguousarray(out.reshape(B, S, OF).astype(np.float32))


def kernel(x, bp, scale):
    in_maps = make_in_maps(x, bp, scale)
    nc = _get_nc()
    res = run_bass_kernel_spmd(nc, in_maps, core_ids=list(range(NCORES)))
    return gather(res.results)


if __name__ == "__main__":
    rng = np.random.default_rng(0)
    x = rng.standard_normal((B, S, IF), dtype=np.float32)
    bp = rng.integers(0, 256, size=(OF * IF // 8,), dtype=np.int32)
    scale = np.ones((1,), dtype=np.float32)
    out = kernel(x=x, bp=bp, scale=scale)
    print(out.shape, out.dtype)
